# revision 13
# baseline (speedup 1.0000x reference)
"""PrRoIPool2D (precise ROI pooling) Trainium2 kernel — 8-core SPMD.

Strategy ("fused banded sweep"):
  out[r,c,p,q] = sum_{h,w} F[b_r,c,h,w] * Iy[r,p,h] * Ix[r,q,w]
The (Iy ⊗ Ix) basis is banded: bin (r,p) touches only a ~5-row window of h.
Host packs, per core (= one feature batch), a basis tensor B whose columns are
(r,p,q) output columns placed at GLOBALLY RANK-ALIGNED slots (all cores'
windows merged and sorted by h-window start; each window's slot ≈ its global
rank / 8).  This keeps every core's alive-column interval for a given 2-row
h-chunk at nearly the same slot range, so the cross-core union interval
[LO_k, HI_k) that must be baked into the shared SPMD program stays tight
(~2.4x smaller NB than per-core sorting with padding).

For each 2-row h-chunk k the device runs one matmul per (chunk, c-half,
psum-bank-piece) with the features as stationary weights, PSUM-accumulating
straight into the final output columns (per-element has_written semantics make
first-write overwrite, later writes add).  Banks are evacuated (fp32->bf16)
eagerly as soon as their last chunk has accumulated, and each c-half's output
is shipped to DRAM with a single wide DMA so output overlaps compute.

Everything F-dependent runs on device; the host only does O(R*(H+W))
coordinate preprocessing (tent-basis integrals, slot assignment, packing) and
output unpermutation.
"""

import numpy as np
import ml_dtypes

POOLED = 7
SCALE = 0.5
N, C, H, W = 8, 256, 56, 56
NCORES = 8
CHUNK_H = 2
NCHUNK = H // CHUNK_H          # 28
KDIM = CHUNK_H * W             # 112 (payload rows)
KPAD = 128                     # device K rows (padded for fast weight load)
SIM_SAFE = False               # True: split MMs for CoreSim's uniformity assert
BANK = 512                     # fp32 elements per PSUM bank
BF16 = ml_dtypes.bfloat16

_kernel_cache = {}
LAST_RESULTS = None            # BassKernelResults stash for test harnesses


def _tent_integral(start, end, n):
    i = np.arange(n, dtype=np.float64)
    a = np.clip(start[..., None] - i, -1.0, 1.0)
    b = np.clip(end[..., None] - i, -1.0, 1.0)

    def G(t):
        return np.where(t <= 0.0, 0.5 * (t + 1.0) ** 2, 1.0 - 0.5 * (1.0 - t) ** 2)

    return G(b) - G(a)


def _host_prep(features, rois):
    """Build per-core packed device inputs + unpack metadata."""
    R = rois.shape[0]
    batch = rois[:, 0].astype(np.int32)
    x1 = rois[:, 1].astype(np.float64) * SCALE
    y1 = rois[:, 2].astype(np.float64) * SCALE
    x2 = rois[:, 3].astype(np.float64) * SCALE
    y2 = rois[:, 4].astype(np.float64) * SCALE
    bw = (x2 - x1) / POOLED
    bh = (y2 - y1) / POOLED
    pw = np.arange(POOLED, dtype=np.float64)
    xs = x1[:, None] + pw * bw[:, None]
    ys = y1[:, None] + pw * bh[:, None]
    Ix = _tent_integral(xs, xs + bw[:, None], W)       # [R,7,W]
    Iy = _tent_integral(ys, ys + bh[:, None], H)       # [R,7,H]
    area = bw * bh
    scl = np.where(area > 0, 1.0 / np.maximum(area, 1e-12), 0.0)
    Iy_s = Iy * scl[:, None, None]

    core_rois = [np.nonzero(batch == c)[0] for c in range(NCORES)]

    # per-core (lo, hi, rg, p) h-windows, sorted by window start
    core_wins = []
    for c in range(NCORES):
        wins = []
        for rg in core_rois[c]:
            for p in range(POOLED):
                nz = np.nonzero(Iy_s[rg, p] != 0)[0]
                lo, hi = (int(nz[0]), int(nz[-1])) if len(nz) else (0, 0)
                wins.append((lo, hi, int(rg), p))
        wins.sort(key=lambda t: (t[0], t[1]))
        core_wins.append(wins)

    # Slot assignment: minimize sum_k of cross-core union interval widths.
    # Init with global rank alignment, then anneal with a per-core DP
    # (order-preserving, strictly increasing slots) against the other cores'
    # union shrunk by delta — the shrink pressure escapes the local optimum
    # where every core is individually tight but collectively misaligned.
    merged = sorted(
        (w[0], w[1], c, i)
        for c, wins in enumerate(core_wins)
        for i, w in enumerate(wins)
    )
    slot = [[0] * len(core_wins[c]) for c in range(NCORES)]
    nxt = [0] * NCORES
    for r, (_, _, c, i) in enumerate(merged):
        s = max(nxt[c], r // NCORES)
        slot[c][i] = s
        nxt[c] = s + 1
    S = max(nxt) + 30

    def core_ab(wins):
        """Per-chunk (min, max) alive window index (birth-sorted), or None."""
        ab = []
        for k in range(NCHUNK):
            h0, h1 = CHUNK_H * k, CHUNK_H * k + CHUNK_H - 1
            alive = [i for i, (lo, hi, _, _) in enumerate(wins)
                     if lo <= h1 and hi >= h0]
            ab.append((alive[0], alive[-1]) if alive else None)
        return ab

    abs_ = [core_ab(w) for w in core_wins]

    def dp_core(wins, ab, LOs, HIs):
        """Optimal increasing slot assignment vs target intervals [LOs,HIs)."""
        n = len(wins)
        Ach = [[] for _ in range(n)]
        Bch = [[] for _ in range(n)]
        for k, x in enumerate(ab):
            if x is None:
                continue
            a, b = x
            Ach[a].append(k)
            Bch[b].append(k)
        srange = np.arange(S)
        INF = 1e18
        cost = np.zeros((n, S))
        for i in range(n):
            ci = np.zeros(S)
            for k in Ach[i]:
                ci += np.maximum(0, LOs[k] - srange)
            for k in Bch[i]:
                ci += np.maximum(0, srange + 1 - HIs[k])
            cost[i] = ci
        f = cost[0].copy()
        parents = np.zeros((n, S), dtype=np.int32)
        for i in range(1, n):
            pm = np.minimum.accumulate(f)
            am = np.where(f <= pm, srange, 0)
            am = np.maximum.accumulate(am)
            g = np.full(S, INF)
            g[i:] = pm[i - 1:S - 1] + cost[i][i:]
            parents[i, 1:] = am[:S - 1]
            f = g
        s = int(np.argmin(f))
        out = [0] * n
        for i in range(n - 1, -1, -1):
            out[i] = s
            if i > 0:
                s = int(parents[i][s])
        return out

    def unions(slots, excl=-1):
        LO = np.full(NCHUNK, 10 ** 9, dtype=np.int64)
        HI = np.zeros(NCHUNK, dtype=np.int64)
        for c in range(NCORES):
            if c == excl:
                continue
            for k, x in enumerate(abs_[c]):
                if x is None:
                    continue
                a, b = x
                LO[k] = min(LO[k], slots[c][a])
                HI[k] = max(HI[k], slots[c][b] + 1)
        return LO, HI

    best = None
    for delta in (6, 5, 4, 4, 3, 3, 2, 2, 2, 1, 1, 1, 0, 0, 2, 1, 0, 0):
        for c in range(NCORES):
            LOx, HIx = unions(slot, excl=c)
            slot[c] = dp_core(core_wins[c], abs_[c], LOx + delta,
                              np.maximum(HIx - delta, LOx + delta + 1))
        LOu, HIu = unions(slot)
        nb = int(sum(HIu[k] - LOu[k] for k in range(NCHUNK) if HIu[k] > 0))
        if best is None or nb < best[0]:
            best = (nb, [list(s) for s in slot])
    slot = best[1]
    NGRP = max(max(s) for s in slot if s) + 1

    # per-chunk alive slot interval (union over cores)
    LO = np.full(NCHUNK, NGRP, dtype=np.int64)
    HI = np.zeros(NCHUNK, dtype=np.int64)
    for c in range(NCORES):
        wins = core_wins[c]
        if not wins:
            continue
        lo_arr = np.array([w[0] for w in wins])
        hi_arr = np.array([w[1] for w in wins])
        sl_arr = np.array(slot[c])
        for k in range(NCHUNK):
            h0, h1 = CHUNK_H * k, CHUNK_H * k + CHUNK_H - 1
            alive = np.nonzero((lo_arr <= h1) & (hi_arr >= h0))[0]
            if len(alive):
                LO[k] = min(LO[k], int(sl_arr[alive].min()))
                HI[k] = max(HI[k], int(sl_arr[alive].max()) + 1)
    active = HI > 0
    LOc, HIc = LO * POOLED, HI * POOLED
    COLS = max(int(HIc[k]) for k in range(NCHUNK) if active[k])
    NBANK = (COLS + BANK - 1) // BANK

    offs = np.zeros(NCHUNK + 1, dtype=np.int64)
    for k in range(NCHUNK):
        offs[k + 1] = offs[k] + (int(HIc[k] - LOc[k]) if active[k] else 0)
    NB = int(offs[-1])

    # pack B (bf16) per core: B[(dh,w), packed_col]
    B = np.zeros((NCORES, KDIM, NB), dtype=np.float32)
    IxT = Ix.transpose(0, 2, 1)                        # [R, W, 7]
    for c in range(NCORES):
        wins = core_wins[c]
        for i, (wlo, whi, rg, p) in enumerate(wins):
            s = slot[c][i]
            for k in range(max(0, (wlo - 1) // CHUNK_H),
                           min(NCHUNK, whi // CHUNK_H + 1)):
                if not active[k]:
                    continue
                if not (LO[k] <= s < HI[k]):
                    continue
                h0, h1 = CHUNK_H * k, CHUNK_H * k + CHUNK_H - 1
                if wlo > h1 or whi < h0:
                    continue
                cb = int(offs[k]) + (s * POOLED - int(LOc[k]))
                for dh in range(CHUNK_H):
                    h = CHUNK_H * k + dh
                    if wlo <= h <= whi:
                        B[c, dh * W:(dh + 1) * W, cb:cb + POOLED] = (
                            Iy_s[rg, p, h] * IxT[rg]
                        )
    B = np.pad(B, ((0, 0), (0, KPAD - KDIM), (0, 0))).astype(BF16)

    # features per core, chunk-major transposed: FT[(dh,w), k*C + cc]
    f = features.astype(np.float32)                    # [N,C,H,W]
    # [N, C, k, dh, w] -> [N, dh, w, k, C]
    ft = f.reshape(N, C, NCHUNK, CHUNK_H, W).transpose(0, 3, 4, 2, 1)
    FT = np.pad(ft.reshape(N, KDIM, NCHUNK * C),
                ((0, 0), (0, KPAD - KDIM), (0, 0))).astype(BF16)

    return dict(B=B, FT=FT, offs=offs, LOc=LOc.astype(int), HIc=HIc.astype(int),
                active=active, core_wins=core_wins, slot=slot, COLS=COLS,
                NBANK=NBANK, NB=NB, R=R)


def _build_bass(shape_key):
    """Build + compile the SPMD Bass program for given packing metadata."""
    NB, COLS, NBANK, LOc, HIc, active_t, offs = shape_key
    LOc, HIc, active, offs = list(LOc), list(HIc), list(active_t), list(offs)

    import concourse.bass as bass  # noqa: F401
    import concourse.tile as tile
    from concourse import bacc, mybir

    nc = bacc.Bacc("TRN2", target_bir_lowering=False, debug=False,
                   enable_asserts=False, num_devices=NCORES)
    bf = mybir.dt.bfloat16
    f32 = mybir.dt.float32
    ft_ap = nc.dram_tensor("ft", [KPAD, NCHUNK * C], bf, kind="ExternalInput").ap()
    b_ap = nc.dram_tensor("bb", [KPAD, NB], bf, kind="ExternalInput").ap()
    out_ap = nc.dram_tensor("out", [C, COLS], bf, kind="ExternalOutput").ap()

    kact = [k for k in range(NCHUNK) if active[k]]
    # last chunk touching each bank (per-bank stop flag)
    last_k = {}
    for k in kact:
        for bk in range(LOc[k] // BANK, (HIc[k] - 1) // BANK + 1):
            last_k[bk] = k

    with tile.TileContext(nc) as tc:
        with (
            tc.tile_pool(name="ftp", bufs=1) as ftp,
            tc.tile_pool(name="bp", bufs=1) as bp,
            tc.tile_pool(name="pp", bufs=1, space="PSUM") as pp,
            tc.tile_pool(name="op", bufs=2) as op,
        ):
            ft_sb = ftp.tile([KPAD, NCHUNK * C], bf)
            b_sb = bp.tile([KPAD, NB], bf)
            # geometric input splits: a tiny first split lets chunk-0 matmuls
            # start as soon as possible, later fatter splits amortize DGE cost
            # and stream ahead of matmul consumption.  FT on sync queue, B on
            # scalar queue (the two HWDGE engines).
            SPLITS = [0, 2, 5, 10, 17, NCHUNK]
            for k0, k1 in zip(SPLITS[:-1], SPLITS[1:]):
                nc.sync.dma_start(ft_sb[:, k0 * C:k1 * C], ft_ap[:, k0 * C:k1 * C])
                o0, o1 = offs[k0], offs[k1]
                if o1 > o0:
                    nc.scalar.dma_start(b_sb[:, o0:o1], b_ap[:, o0:o1])

            # both c-halves' PSUM banks live simultaneously (2*NBANK <= 8);
            # the two halves interleave per chunk so the tensor engine has 2x
            # work per arrived chunk while the input is still streaming.
            ptiles = [[pp.tile([128, BANK], f32, tag=f"bank{m}_{i}",
                               name=f"pt{m}_{i}") for i in range(NBANK)]
                      for m in range(2)]
            out_sb = [op.tile([128, COLS], bf, name=f"os{m}") for m in range(2)]
            whi = [[-1] * NBANK for _ in range(2)]
            for ki, k in enumerate(kact):
                last = ki == len(kact) - 1
                lo, hi, ob = LOc[k], HIc[k], offs[k]
                for m in range(2):
                    lhsT = ft_sb[:KDIM, k * C + m * 128: k * C + (m + 1) * 128]
                    for bk in range(lo // BANK, (hi - 1) // BANK + 1):
                        s = max(lo, bk * BANK)
                        e = min(hi, (bk + 1) * BANK)
                        is_last = k == last_k[bk]
                        # per-element has_written semantics: first write with
                        # start=True resets, later writes (start=False) add;
                        # stop is sim-only bookkeeping
                        if whi[m][bk] < 0:
                            pieces = [(s, e, True)]
                        elif SIM_SAFE:
                            pieces = []
                            if s < whi[m][bk]:
                                pieces.append((s, min(e, whi[m][bk]), False))
                            if e > whi[m][bk]:
                                pieces.append((max(s, whi[m][bk]), e, False))
                        else:
                            pieces = [(s, e, False)]
                        for pi, (ps, pe, st) in enumerate(pieces):
                            nc.tensor.matmul(
                                ptiles[m][bk][:, ps - bk * BANK: pe - bk * BANK],
                                lhsT=lhsT,
                                rhs=b_sb[:KDIM, ob + ps - lo: ob + pe - lo],
                                start=st,
                                stop=is_last and pi == len(pieces) - 1,
                            )
                        whi[m][bk] = max(whi[m][bk], e)
                # eager evacuation (m=0 via vector+sync, m=1 via scalar, in
                # parallel): banks finishing before the final chunk are
                # copied+shipped whole at completion; banks the final chunk
                # writes are pre-copied (everything outside the final chunk's
                # interval) one chunk early, so the critical tail is only the
                # final interval's small copy + one DMA per bank.
                kl = kact[-1]
                flo, fhi = LOc[kl], HIc[kl]

                def copy_strip(m, a, b):
                    if a >= b:
                        return
                    bk = a // BANK
                    dst = out_sb[m][:, a:b]
                    src = ptiles[m][bk][:, a - bk * BANK: b - bk * BANK]
                    if m == 0:
                        nc.vector.tensor_copy(dst, src)
                    else:
                        nc.scalar.copy(dst, src)

                def ship(m, a, b):
                    eng = nc.sync if m == 0 else nc.scalar
                    eng.dma_start(out_ap[m * 128:(m + 1) * 128, a:b],
                                  out_sb[m][:, a:b])

                for m in range(2):
                    for bk in range(NBANK):
                        b0, b1 = bk * BANK, min((bk + 1) * BANK, COLS)
                        if last_k.get(bk) != k:
                            continue
                        if k != kl:
                            copy_strip(m, b0, b1)
                            ship(m, b0, b1)
                    if ki == len(kact) - 2:
                        # pre-copy final-chunk banks outside [flo, fhi)
                        for bk in range(NBANK):
                            if last_k.get(bk) != kl:
                                continue
                            b0, b1 = bk * BANK, min((bk + 1) * BANK, COLS)
                            copy_strip(m, b0, min(flo, b1))
                            copy_strip(m, max(fhi, b0), b1)
                    if last:
                        for bk in range(NBANK):
                            if last_k.get(bk) != kl:
                                continue
                            b0, b1 = bk * BANK, min((bk + 1) * BANK, COLS)
                            copy_strip(m, max(flo, b0), min(fhi, b1))
                            ship(m, b0, b1)

    nc.compile()
    return nc


def kernel(features, rois):
    global LAST_RESULTS
    from concourse import bass_utils

    features = np.asarray(features, dtype=np.float32)
    rois = np.asarray(rois, dtype=np.float32)
    hp = _host_prep(features, rois)

    shape_key = (hp["NB"], hp["COLS"], hp["NBANK"],
                 tuple(hp["LOc"]), tuple(hp["HIc"]),
                 tuple(bool(a) for a in hp["active"]),
                 tuple(int(o) for o in hp["offs"]))
    nc = _kernel_cache.get(shape_key)
    if nc is None:
        nc = _build_bass(shape_key)
        _kernel_cache[shape_key] = nc

    in_maps = [{"ft": np.ascontiguousarray(hp["FT"][c]),
                "bb": np.ascontiguousarray(hp["B"][c])}
               for c in range(NCORES)]
    res = bass_utils.run_bass_kernel_spmd(nc, in_maps, core_ids=list(range(NCORES)))
    LAST_RESULTS = res

    # unpack: out_core[c_chan, col(slot,q)] -> final[r, c_chan, p, q]
    final = np.zeros((hp["R"], C, POOLED, POOLED), dtype=np.float32)
    for c in range(NCORES):
        out = np.asarray(res.results[c]["out"], dtype=np.float32)  # [C, COLS]
        wins = hp["core_wins"][c]
        if not wins:
            continue
        sl = hp["slot"][c]
        rgs = np.array([w[2] for w in wins])
        ps = np.array([w[3] for w in wins])
        cols = out.reshape(C, -1, POOLED)[:, sl, :]    # [C, nwin, 7]
        final[rgs, :, ps, :] = cols.transpose(1, 0, 2)
    return final


# revision 14
# speedup vs baseline: 1.0891x; 1.0891x over previous
"""PrRoIPool2D (precise ROI pooling) Trainium2 kernel — 8-core SPMD.

Strategy ("fused banded sweep"):
  out[r,c,p,q] = sum_{h,w} F[b_r,c,h,w] * Iy[r,p,h] * Ix[r,q,w]
The (Iy ⊗ Ix) basis is banded: bin (r,p) touches only a ~5-row window of h.
Host packs, per core (= one feature batch), a basis tensor B whose columns are
(r,p,q) output columns placed at GLOBALLY RANK-ALIGNED slots (all cores'
windows merged and sorted by h-window start; each window's slot ≈ its global
rank / 8).  This keeps every core's alive-column interval for a given 2-row
h-chunk at nearly the same slot range, so the cross-core union interval
[LO_k, HI_k) that must be baked into the shared SPMD program stays tight
(~2.4x smaller NB than per-core sorting with padding).

For each 2-row h-chunk k the device runs one matmul per (chunk, c-half,
psum-bank-piece) with the features as stationary weights, PSUM-accumulating
straight into the final output columns (per-element has_written semantics make
first-write overwrite, later writes add).  Banks are evacuated (fp32->bf16)
eagerly as soon as their last chunk has accumulated, and each c-half's output
is shipped to DRAM with a single wide DMA so output overlaps compute.

Everything F-dependent runs on device; the host only does O(R*(H+W))
coordinate preprocessing (tent-basis integrals, slot assignment, packing) and
output unpermutation.
"""

import numpy as np
import ml_dtypes

POOLED = 7
SCALE = 0.5
N, C, H, W = 8, 256, 56, 56
NCORES = 8
CHUNK_H = 2
NCHUNK = H // CHUNK_H          # 28
KDIM = CHUNK_H * W             # 112 (payload rows)
KPAD = 128                     # device K rows (padded for fast weight load)
SIM_SAFE = False               # True: split MMs for CoreSim's uniformity assert
BANK = 512                     # fp32 elements per PSUM bank
BF16 = ml_dtypes.bfloat16

_kernel_cache = {}
LAST_RESULTS = None            # BassKernelResults stash for test harnesses


def _tent_integral(start, end, n):
    i = np.arange(n, dtype=np.float64)
    a = np.clip(start[..., None] - i, -1.0, 1.0)
    b = np.clip(end[..., None] - i, -1.0, 1.0)

    def G(t):
        return np.where(t <= 0.0, 0.5 * (t + 1.0) ** 2, 1.0 - 0.5 * (1.0 - t) ** 2)

    return G(b) - G(a)


def _host_prep(features, rois):
    """Build per-core packed device inputs + unpack metadata."""
    R = rois.shape[0]
    batch = rois[:, 0].astype(np.int32)
    x1 = rois[:, 1].astype(np.float64) * SCALE
    y1 = rois[:, 2].astype(np.float64) * SCALE
    x2 = rois[:, 3].astype(np.float64) * SCALE
    y2 = rois[:, 4].astype(np.float64) * SCALE
    bw = (x2 - x1) / POOLED
    bh = (y2 - y1) / POOLED
    pw = np.arange(POOLED, dtype=np.float64)
    xs = x1[:, None] + pw * bw[:, None]
    ys = y1[:, None] + pw * bh[:, None]
    Ix = _tent_integral(xs, xs + bw[:, None], W)       # [R,7,W]
    Iy = _tent_integral(ys, ys + bh[:, None], H)       # [R,7,H]
    area = bw * bh
    scl = np.where(area > 0, 1.0 / np.maximum(area, 1e-12), 0.0)
    Iy_s = Iy * scl[:, None, None]

    core_rois = [np.nonzero(batch == c)[0] for c in range(NCORES)]

    # per-core (lo, hi, rg, p) h-windows, sorted by window start
    core_wins = []
    for c in range(NCORES):
        wins = []
        for rg in core_rois[c]:
            for p in range(POOLED):
                nz = np.nonzero(Iy_s[rg, p] != 0)[0]
                lo, hi = (int(nz[0]), int(nz[-1])) if len(nz) else (0, 0)
                wins.append((lo, hi, int(rg), p))
        wins.sort(key=lambda t: (t[0], t[1]))
        core_wins.append(wins)

    # Slot assignment: minimize sum_k of cross-core union interval widths.
    # Init with global rank alignment, then anneal with a per-core DP
    # (order-preserving, strictly increasing slots) against the other cores'
    # union shrunk by delta — the shrink pressure escapes the local optimum
    # where every core is individually tight but collectively misaligned.
    merged = sorted(
        (w[0], w[1], c, i)
        for c, wins in enumerate(core_wins)
        for i, w in enumerate(wins)
    )
    slot = [[0] * len(core_wins[c]) for c in range(NCORES)]
    nxt = [0] * NCORES
    for r, (_, _, c, i) in enumerate(merged):
        s = max(nxt[c], r // NCORES)
        slot[c][i] = s
        nxt[c] = s + 1
    S = max(nxt) + 30

    def core_ab(wins):
        """Per-chunk (min, max) alive window index (birth-sorted), or None."""
        ab = []
        for k in range(NCHUNK):
            h0, h1 = CHUNK_H * k, CHUNK_H * k + CHUNK_H - 1
            alive = [i for i, (lo, hi, _, _) in enumerate(wins)
                     if lo <= h1 and hi >= h0]
            ab.append((alive[0], alive[-1]) if alive else None)
        return ab

    abs_ = [core_ab(w) for w in core_wins]

    def dp_core(wins, ab, LOs, HIs):
        """Optimal increasing slot assignment vs target intervals [LOs,HIs)."""
        n = len(wins)
        Ach = [[] for _ in range(n)]
        Bch = [[] for _ in range(n)]
        for k, x in enumerate(ab):
            if x is None:
                continue
            a, b = x
            Ach[a].append(k)
            Bch[b].append(k)
        srange = np.arange(S)
        INF = 1e18
        cost = np.zeros((n, S))
        for i in range(n):
            ci = np.zeros(S)
            for k in Ach[i]:
                ci += np.maximum(0, LOs[k] - srange)
            for k in Bch[i]:
                ci += np.maximum(0, srange + 1 - HIs[k])
            cost[i] = ci
        f = cost[0].copy()
        parents = np.zeros((n, S), dtype=np.int32)
        for i in range(1, n):
            pm = np.minimum.accumulate(f)
            am = np.where(f <= pm, srange, 0)
            am = np.maximum.accumulate(am)
            g = np.full(S, INF)
            g[i:] = pm[i - 1:S - 1] + cost[i][i:]
            parents[i, 1:] = am[:S - 1]
            f = g
        s = int(np.argmin(f))
        out = [0] * n
        for i in range(n - 1, -1, -1):
            out[i] = s
            if i > 0:
                s = int(parents[i][s])
        return out

    def unions(slots, excl=-1):
        LO = np.full(NCHUNK, 10 ** 9, dtype=np.int64)
        HI = np.zeros(NCHUNK, dtype=np.int64)
        for c in range(NCORES):
            if c == excl:
                continue
            for k, x in enumerate(abs_[c]):
                if x is None:
                    continue
                a, b = x
                LO[k] = min(LO[k], slots[c][a])
                HI[k] = max(HI[k], slots[c][b] + 1)
        return LO, HI

    best = None
    for delta in (6, 5, 4, 4, 3, 3, 2, 2, 2, 1, 1, 1, 0, 0, 2, 1, 0, 0):
        for c in range(NCORES):
            LOx, HIx = unions(slot, excl=c)
            slot[c] = dp_core(core_wins[c], abs_[c], LOx + delta,
                              np.maximum(HIx - delta, LOx + delta + 1))
        LOu, HIu = unions(slot)
        nb = int(sum(HIu[k] - LOu[k] for k in range(NCHUNK) if HIu[k] > 0))
        if best is None or nb < best[0]:
            best = (nb, [list(s) for s in slot])
    slot = best[1]
    NGRP = max(max(s) for s in slot if s) + 1

    # per-chunk alive slot interval (union over cores)
    LO = np.full(NCHUNK, NGRP, dtype=np.int64)
    HI = np.zeros(NCHUNK, dtype=np.int64)
    for c in range(NCORES):
        wins = core_wins[c]
        if not wins:
            continue
        lo_arr = np.array([w[0] for w in wins])
        hi_arr = np.array([w[1] for w in wins])
        sl_arr = np.array(slot[c])
        for k in range(NCHUNK):
            h0, h1 = CHUNK_H * k, CHUNK_H * k + CHUNK_H - 1
            alive = np.nonzero((lo_arr <= h1) & (hi_arr >= h0))[0]
            if len(alive):
                LO[k] = min(LO[k], int(sl_arr[alive].min()))
                HI[k] = max(HI[k], int(sl_arr[alive].max()) + 1)
    active = HI > 0
    LOc, HIc = LO * POOLED, HI * POOLED
    COLS = max(int(HIc[k]) for k in range(NCHUNK) if active[k])
    NBANK = (COLS + BANK - 1) // BANK

    offs = np.zeros(NCHUNK + 1, dtype=np.int64)
    for k in range(NCHUNK):
        offs[k + 1] = offs[k] + (int(HIc[k] - LOc[k]) if active[k] else 0)
    NB = int(offs[-1])

    # pack B (bf16) per core: B[(dh,w), packed_col]
    B = np.zeros((NCORES, KDIM, NB), dtype=np.float32)
    IxT = Ix.transpose(0, 2, 1)                        # [R, W, 7]
    for c in range(NCORES):
        wins = core_wins[c]
        for i, (wlo, whi, rg, p) in enumerate(wins):
            s = slot[c][i]
            for k in range(max(0, (wlo - 1) // CHUNK_H),
                           min(NCHUNK, whi // CHUNK_H + 1)):
                if not active[k]:
                    continue
                if not (LO[k] <= s < HI[k]):
                    continue
                h0, h1 = CHUNK_H * k, CHUNK_H * k + CHUNK_H - 1
                if wlo > h1 or whi < h0:
                    continue
                cb = int(offs[k]) + (s * POOLED - int(LOc[k]))
                for dh in range(CHUNK_H):
                    h = CHUNK_H * k + dh
                    if wlo <= h <= whi:
                        B[c, dh * W:(dh + 1) * W, cb:cb + POOLED] = (
                            Iy_s[rg, p, h] * IxT[rg]
                        )
    B = np.pad(B, ((0, 0), (0, KPAD - KDIM), (0, 0))).astype(BF16)

    # features per core, chunk-major transposed: FT[(dh,w), k*C + cc]
    f = features.astype(np.float32)                    # [N,C,H,W]
    # [N, C, k, dh, w] -> [N, dh, w, k, C]
    ft = f.reshape(N, C, NCHUNK, CHUNK_H, W).transpose(0, 3, 4, 2, 1)
    FT = np.pad(ft.reshape(N, KDIM, NCHUNK * C),
                ((0, 0), (0, KPAD - KDIM), (0, 0))).astype(BF16)

    return dict(B=B, FT=FT, offs=offs, LOc=LOc.astype(int), HIc=HIc.astype(int),
                active=active, core_wins=core_wins, slot=slot, COLS=COLS,
                NBANK=NBANK, NB=NB, R=R)


def _build_bass(shape_key):
    """Build + compile the SPMD Bass program for given packing metadata."""
    NB, COLS, NBANK, LOc, HIc, active_t, offs = shape_key
    LOc, HIc, active, offs = list(LOc), list(HIc), list(active_t), list(offs)

    import concourse.bass as bass  # noqa: F401
    import concourse.tile as tile
    from concourse import bacc, mybir

    nc = bacc.Bacc("TRN2", target_bir_lowering=False, debug=False,
                   enable_asserts=False, num_devices=NCORES)
    bf = mybir.dt.bfloat16
    f32 = mybir.dt.float32
    ft_ap = nc.dram_tensor("ft", [KPAD, NCHUNK * C], bf, kind="ExternalInput").ap()
    b_ap = nc.dram_tensor("bb", [KPAD, NB], bf, kind="ExternalInput").ap()
    out_ap = nc.dram_tensor("out", [C, COLS], bf, kind="ExternalOutput").ap()

    kact = [k for k in range(NCHUNK) if active[k]]
    # last chunk touching each bank (per-bank stop flag)
    last_k = {}
    for k in kact:
        for bk in range(LOc[k] // BANK, (HIc[k] - 1) // BANK + 1):
            last_k[bk] = k

    with tile.TileContext(nc) as tc:
        with (
            tc.tile_pool(name="ftp", bufs=1) as ftp,
            tc.tile_pool(name="bp", bufs=1) as bp,
            tc.tile_pool(name="pp", bufs=1, space="PSUM") as pp,
            tc.tile_pool(name="op", bufs=2) as op,
        ):
            ft_sb = ftp.tile([KPAD, NCHUNK * C], bf)
            b_sb = bp.tile([KPAD, NB], bf)
            # geometric input splits: a tiny first split lets chunk-0 matmuls
            # start as soon as possible, later fatter splits amortize DGE cost
            # and stream ahead of matmul consumption.  FT on sync queue, B on
            # scalar queue (the two HWDGE engines).
            SPLITS = [0, 2, 5, 10, 17, NCHUNK]
            for k0, k1 in zip(SPLITS[:-1], SPLITS[1:]):
                nc.sync.dma_start(ft_sb[:, k0 * C:k1 * C], ft_ap[:, k0 * C:k1 * C])
                o0, o1 = offs[k0], offs[k1]
                if o1 > o0:
                    nc.scalar.dma_start(b_sb[:, o0:o1], b_ap[:, o0:o1])

            # both c-halves' PSUM banks live simultaneously (2*NBANK <= 8);
            # the two halves interleave per chunk so the tensor engine has 2x
            # work per arrived chunk while the input is still streaming.
            ptiles = [[pp.tile([128, BANK], f32, tag=f"bank{m}_{i}",
                               name=f"pt{m}_{i}") for i in range(NBANK)]
                      for m in range(2)]
            out_sb = [op.tile([128, COLS], bf, name=f"os{m}") for m in range(2)]
            whi = [[-1] * NBANK for _ in range(2)]
            for ki, k in enumerate(kact):
                last = ki == len(kact) - 1
                lo, hi, ob = LOc[k], HIc[k], offs[k]
                for m in range(2):
                    lhsT = ft_sb[:KDIM, k * C + m * 128: k * C + (m + 1) * 128]
                    for bk in range(lo // BANK, (hi - 1) // BANK + 1):
                        s = max(lo, bk * BANK)
                        e = min(hi, (bk + 1) * BANK)
                        is_last = k == last_k[bk]
                        # per-element has_written semantics: first write with
                        # start=True resets, later writes (start=False) add;
                        # stop is sim-only bookkeeping
                        if whi[m][bk] < 0:
                            pieces = [(s, e, True)]
                        elif SIM_SAFE:
                            pieces = []
                            if s < whi[m][bk]:
                                pieces.append((s, min(e, whi[m][bk]), False))
                            if e > whi[m][bk]:
                                pieces.append((max(s, whi[m][bk]), e, False))
                        else:
                            pieces = [(s, e, False)]
                        for pi, (ps, pe, st) in enumerate(pieces):
                            nc.tensor.matmul(
                                ptiles[m][bk][:, ps - bk * BANK: pe - bk * BANK],
                                lhsT=lhsT,
                                rhs=b_sb[:KDIM, ob + ps - lo: ob + pe - lo],
                                start=st,
                                stop=is_last and pi == len(pieces) - 1,
                            )
                        whi[m][bk] = max(whi[m][bk], e)
                # eager per-bank evacuation (m=0 via vector+sync, m=1 via
                # scalar): each PSUM tile is read exactly once, right after
                # its last accumulating chunk — no read ever blocks a later
                # matmul, and output DMA overlaps the remaining compute.
                for m in range(2):
                    for bk in range(NBANK):
                        if last_k.get(bk) != k:
                            continue
                        b0, b1 = bk * BANK, min((bk + 1) * BANK, COLS)
                        dst = out_sb[m][:, b0:b1]
                        if m == 0:
                            nc.vector.tensor_copy(dst, ptiles[m][bk][:, :b1 - b0])
                        else:
                            nc.scalar.copy(dst, ptiles[m][bk][:, :b1 - b0])
                        eng = nc.sync if m == 0 else nc.scalar
                        eng.dma_start(out_ap[m * 128:(m + 1) * 128, b0:b1], dst)

    nc.compile()
    return nc


def kernel(features, rois):
    global LAST_RESULTS
    from concourse import bass_utils

    features = np.asarray(features, dtype=np.float32)
    rois = np.asarray(rois, dtype=np.float32)
    hp = _host_prep(features, rois)

    shape_key = (hp["NB"], hp["COLS"], hp["NBANK"],
                 tuple(hp["LOc"]), tuple(hp["HIc"]),
                 tuple(bool(a) for a in hp["active"]),
                 tuple(int(o) for o in hp["offs"]))
    nc = _kernel_cache.get(shape_key)
    if nc is None:
        nc = _build_bass(shape_key)
        _kernel_cache[shape_key] = nc

    in_maps = [{"ft": np.ascontiguousarray(hp["FT"][c]),
                "bb": np.ascontiguousarray(hp["B"][c])}
               for c in range(NCORES)]
    res = bass_utils.run_bass_kernel_spmd(nc, in_maps, core_ids=list(range(NCORES)))
    LAST_RESULTS = res

    # unpack: out_core[c_chan, col(slot,q)] -> final[r, c_chan, p, q]
    final = np.zeros((hp["R"], C, POOLED, POOLED), dtype=np.float32)
    for c in range(NCORES):
        out = np.asarray(res.results[c]["out"], dtype=np.float32)  # [C, COLS]
        wins = hp["core_wins"][c]
        if not wins:
            continue
        sl = hp["slot"][c]
        rgs = np.array([w[2] for w in wins])
        ps = np.array([w[3] for w in wins])
        cols = out.reshape(C, -1, POOLED)[:, sl, :]    # [C, nwin, 7]
        final[rgs, :, ps, :] = cols.transpose(1, 0, 2)
    return final


# revision 15
# speedup vs baseline: 1.1409x; 1.0476x over previous
"""PrRoIPool2D (precise ROI pooling) Trainium2 kernel — 8-core SPMD.

Strategy ("fused banded sweep"):
  out[r,c,p,q] = sum_{h,w} F[b_r,c,h,w] * Iy[r,p,h] * Ix[r,q,w]
The (Iy ⊗ Ix) basis is banded: bin (r,p) touches only a ~5-row window of h.
Host packs, per core (= one feature batch), a basis tensor B whose columns are
(r,p,q) output columns placed at GLOBALLY RANK-ALIGNED slots (all cores'
windows merged and sorted by h-window start; each window's slot ≈ its global
rank / 8).  This keeps every core's alive-column interval for a given 2-row
h-chunk at nearly the same slot range, so the cross-core union interval
[LO_k, HI_k) that must be baked into the shared SPMD program stays tight
(~2.4x smaller NB than per-core sorting with padding).

For each 2-row h-chunk k the device runs one matmul per (chunk, c-half,
psum-bank-piece) with the features as stationary weights, PSUM-accumulating
straight into the final output columns (per-element has_written semantics make
first-write overwrite, later writes add).  Banks are evacuated (fp32->bf16)
eagerly as soon as their last chunk has accumulated, and each c-half's output
is shipped to DRAM with a single wide DMA so output overlaps compute.

Everything F-dependent runs on device; the host only does O(R*(H+W))
coordinate preprocessing (tent-basis integrals, slot assignment, packing) and
output unpermutation.
"""

import numpy as np
import ml_dtypes

POOLED = 7
SCALE = 0.5
N, C, H, W = 8, 256, 56, 56
NCORES = 8
CHUNK_H = 2
NCHUNK = H // CHUNK_H          # 28
KDIM = CHUNK_H * W             # 112 (payload rows)
KPAD = 128                     # device K rows (padded for fast weight load)
SIM_SAFE = False               # True: split MMs for CoreSim's uniformity assert
BANK = 512                     # fp32 elements per PSUM bank
BF16 = ml_dtypes.bfloat16

_kernel_cache = {}
LAST_RESULTS = None            # BassKernelResults stash for test harnesses


def _tent_integral(start, end, n):
    i = np.arange(n, dtype=np.float64)
    a = np.clip(start[..., None] - i, -1.0, 1.0)
    b = np.clip(end[..., None] - i, -1.0, 1.0)

    def G(t):
        return np.where(t <= 0.0, 0.5 * (t + 1.0) ** 2, 1.0 - 0.5 * (1.0 - t) ** 2)

    return G(b) - G(a)


def _host_prep(features, rois):
    """Build per-core packed device inputs + unpack metadata."""
    R = rois.shape[0]
    batch = rois[:, 0].astype(np.int32)
    x1 = rois[:, 1].astype(np.float64) * SCALE
    y1 = rois[:, 2].astype(np.float64) * SCALE
    x2 = rois[:, 3].astype(np.float64) * SCALE
    y2 = rois[:, 4].astype(np.float64) * SCALE
    bw = (x2 - x1) / POOLED
    bh = (y2 - y1) / POOLED
    pw = np.arange(POOLED, dtype=np.float64)
    xs = x1[:, None] + pw * bw[:, None]
    ys = y1[:, None] + pw * bh[:, None]
    Ix = _tent_integral(xs, xs + bw[:, None], W)       # [R,7,W]
    Iy = _tent_integral(ys, ys + bh[:, None], H)       # [R,7,H]
    area = bw * bh
    scl = np.where(area > 0, 1.0 / np.maximum(area, 1e-12), 0.0)
    Iy_s = Iy * scl[:, None, None]

    core_rois = [np.nonzero(batch == c)[0] for c in range(NCORES)]

    # per-core (lo, hi, rg, p) h-windows, sorted by window start
    core_wins = []
    for c in range(NCORES):
        wins = []
        for rg in core_rois[c]:
            for p in range(POOLED):
                nz = np.nonzero(Iy_s[rg, p] != 0)[0]
                lo, hi = (int(nz[0]), int(nz[-1])) if len(nz) else (0, 0)
                wins.append((lo, hi, int(rg), p))
        wins.sort(key=lambda t: (t[0], t[1]))
        core_wins.append(wins)

    # Slot assignment: minimize sum_k of cross-core union interval widths.
    # Init with global rank alignment, then anneal with a per-core DP
    # (order-preserving, strictly increasing slots) against the other cores'
    # union shrunk by delta — the shrink pressure escapes the local optimum
    # where every core is individually tight but collectively misaligned.
    merged = sorted(
        (w[0], w[1], c, i)
        for c, wins in enumerate(core_wins)
        for i, w in enumerate(wins)
    )
    slot = [[0] * len(core_wins[c]) for c in range(NCORES)]
    nxt = [0] * NCORES
    for r, (_, _, c, i) in enumerate(merged):
        s = max(nxt[c], r // NCORES)
        slot[c][i] = s
        nxt[c] = s + 1
    S = max(nxt) + 30

    def core_ab(wins):
        """Per-chunk (min, max) alive window index (birth-sorted), or None."""
        ab = []
        for k in range(NCHUNK):
            h0, h1 = CHUNK_H * k, CHUNK_H * k + CHUNK_H - 1
            alive = [i for i, (lo, hi, _, _) in enumerate(wins)
                     if lo <= h1 and hi >= h0]
            ab.append((alive[0], alive[-1]) if alive else None)
        return ab

    abs_ = [core_ab(w) for w in core_wins]

    def dp_core(wins, ab, LOs, HIs):
        """Optimal increasing slot assignment vs target intervals [LOs,HIs)."""
        n = len(wins)
        Ach = [[] for _ in range(n)]
        Bch = [[] for _ in range(n)]
        for k, x in enumerate(ab):
            if x is None:
                continue
            a, b = x
            Ach[a].append(k)
            Bch[b].append(k)
        srange = np.arange(S)
        INF = 1e18
        cost = np.zeros((n, S))
        for i in range(n):
            ci = np.zeros(S)
            for k in Ach[i]:
                ci += np.maximum(0, LOs[k] - srange)
            for k in Bch[i]:
                ci += np.maximum(0, srange + 1 - HIs[k])
            cost[i] = ci
        f = cost[0].copy()
        parents = np.zeros((n, S), dtype=np.int32)
        for i in range(1, n):
            pm = np.minimum.accumulate(f)
            am = np.where(f <= pm, srange, 0)
            am = np.maximum.accumulate(am)
            g = np.full(S, INF)
            g[i:] = pm[i - 1:S - 1] + cost[i][i:]
            parents[i, 1:] = am[:S - 1]
            f = g
        s = int(np.argmin(f))
        out = [0] * n
        for i in range(n - 1, -1, -1):
            out[i] = s
            if i > 0:
                s = int(parents[i][s])
        return out

    def unions(slots, excl=-1):
        LO = np.full(NCHUNK, 10 ** 9, dtype=np.int64)
        HI = np.zeros(NCHUNK, dtype=np.int64)
        for c in range(NCORES):
            if c == excl:
                continue
            for k, x in enumerate(abs_[c]):
                if x is None:
                    continue
                a, b = x
                LO[k] = min(LO[k], slots[c][a])
                HI[k] = max(HI[k], slots[c][b] + 1)
        return LO, HI

    best = None
    for delta in (6, 5, 4, 4, 3, 3, 2, 2, 2, 1, 1, 1, 0, 0, 2, 1, 0, 0):
        for c in range(NCORES):
            LOx, HIx = unions(slot, excl=c)
            slot[c] = dp_core(core_wins[c], abs_[c], LOx + delta,
                              np.maximum(HIx - delta, LOx + delta + 1))
        LOu, HIu = unions(slot)
        nb = int(sum(HIu[k] - LOu[k] for k in range(NCHUNK) if HIu[k] > 0))
        if best is None or nb < best[0]:
            best = (nb, [list(s) for s in slot])
    slot = best[1]
    NGRP = max(max(s) for s in slot if s) + 1

    # per-chunk alive slot interval (union over cores)
    LO = np.full(NCHUNK, NGRP, dtype=np.int64)
    HI = np.zeros(NCHUNK, dtype=np.int64)
    for c in range(NCORES):
        wins = core_wins[c]
        if not wins:
            continue
        lo_arr = np.array([w[0] for w in wins])
        hi_arr = np.array([w[1] for w in wins])
        sl_arr = np.array(slot[c])
        for k in range(NCHUNK):
            h0, h1 = CHUNK_H * k, CHUNK_H * k + CHUNK_H - 1
            alive = np.nonzero((lo_arr <= h1) & (hi_arr >= h0))[0]
            if len(alive):
                LO[k] = min(LO[k], int(sl_arr[alive].min()))
                HI[k] = max(HI[k], int(sl_arr[alive].max()) + 1)
    active = HI > 0
    LOc, HIc = LO * POOLED, HI * POOLED
    COLS = max(int(HIc[k]) for k in range(NCHUNK) if active[k])
    NBANK = (COLS + BANK - 1) // BANK

    offs = np.zeros(NCHUNK + 1, dtype=np.int64)
    for k in range(NCHUNK):
        offs[k + 1] = offs[k] + (int(HIc[k] - LOc[k]) if active[k] else 0)
    NB = int(offs[-1])

    # pack B (bf16) per core: B[(dh,w), packed_col]
    B = np.zeros((NCORES, KDIM, NB), dtype=np.float32)
    IxT = Ix.transpose(0, 2, 1)                        # [R, W, 7]
    for c in range(NCORES):
        wins = core_wins[c]
        for i, (wlo, whi, rg, p) in enumerate(wins):
            s = slot[c][i]
            for k in range(max(0, (wlo - 1) // CHUNK_H),
                           min(NCHUNK, whi // CHUNK_H + 1)):
                if not active[k]:
                    continue
                if not (LO[k] <= s < HI[k]):
                    continue
                h0, h1 = CHUNK_H * k, CHUNK_H * k + CHUNK_H - 1
                if wlo > h1 or whi < h0:
                    continue
                cb = int(offs[k]) + (s * POOLED - int(LOc[k]))
                for dh in range(CHUNK_H):
                    h = CHUNK_H * k + dh
                    if wlo <= h <= whi:
                        B[c, dh * W:(dh + 1) * W, cb:cb + POOLED] = (
                            Iy_s[rg, p, h] * IxT[rg]
                        )
    B = np.pad(B, ((0, 0), (0, KPAD - KDIM), (0, 0))).astype(BF16)

    # features per core, chunk-major transposed: FT[(dh,w), k*C + cc]
    f = features.astype(np.float32)                    # [N,C,H,W]
    # [N, C, k, dh, w] -> [N, dh, w, k, C]
    ft = f.reshape(N, C, NCHUNK, CHUNK_H, W).transpose(0, 3, 4, 2, 1)
    FT = np.pad(ft.reshape(N, KDIM, NCHUNK * C),
                ((0, 0), (0, KPAD - KDIM), (0, 0))).astype(BF16)

    return dict(B=B, FT=FT, offs=offs, LOc=LOc.astype(int), HIc=HIc.astype(int),
                active=active, core_wins=core_wins, slot=slot, COLS=COLS,
                NBANK=NBANK, NB=NB, R=R)


def _build_bass(shape_key):
    """Build + compile the SPMD Bass program for given packing metadata."""
    NB, COLS, NBANK, LOc, HIc, active_t, offs = shape_key
    LOc, HIc, active, offs = list(LOc), list(HIc), list(active_t), list(offs)

    import concourse.bass as bass  # noqa: F401
    import concourse.tile as tile
    from concourse import bacc, mybir

    nc = bacc.Bacc("TRN2", target_bir_lowering=False, debug=False,
                   enable_asserts=False, num_devices=NCORES)
    bf = mybir.dt.bfloat16
    f32 = mybir.dt.float32
    ft_ap = nc.dram_tensor("ft", [KPAD, NCHUNK * C], bf, kind="ExternalInput").ap()
    b_ap = nc.dram_tensor("bb", [KPAD, NB], bf, kind="ExternalInput").ap()
    out_ap = nc.dram_tensor("out", [C, COLS], bf, kind="ExternalOutput").ap()

    kact = [k for k in range(NCHUNK) if active[k]]
    # last chunk touching each bank (per-bank stop flag)
    last_k = {}
    for k in kact:
        for bk in range(LOc[k] // BANK, (HIc[k] - 1) // BANK + 1):
            last_k[bk] = k

    with tile.TileContext(nc) as tc:
        with (
            tc.tile_pool(name="ftp", bufs=1) as ftp,
            tc.tile_pool(name="bp", bufs=1) as bp,
            tc.tile_pool(name="pp", bufs=1, space="PSUM") as pp,
            tc.tile_pool(name="op", bufs=2) as op,
        ):
            ft_sb = ftp.tile([KPAD, NCHUNK * C], bf)
            b_sb = bp.tile([KPAD, NB], bf)
            # geometric input splits: a tiny first split lets chunk-0 matmuls
            # start as soon as possible, later fatter splits amortize DGE cost
            # and stream ahead of matmul consumption.  FT on sync queue, B on
            # scalar queue (the two HWDGE engines).
            SPLITS = [0, 2, 5, 9, 13, 17, 21, 24, NCHUNK]
            for k0, k1 in zip(SPLITS[:-1], SPLITS[1:]):
                nc.sync.dma_start(ft_sb[:, k0 * C:k1 * C], ft_ap[:, k0 * C:k1 * C])
                o0, o1 = offs[k0], offs[k1]
                if o1 > o0:
                    nc.scalar.dma_start(b_sb[:, o0:o1], b_ap[:, o0:o1])

            # both c-halves' PSUM banks live simultaneously (2*NBANK <= 8);
            # the two halves interleave per chunk so the tensor engine has 2x
            # work per arrived chunk while the input is still streaming.
            ptiles = [[pp.tile([128, BANK], f32, tag=f"bank{m}_{i}",
                               name=f"pt{m}_{i}") for i in range(NBANK)]
                      for m in range(2)]
            out_sb = [op.tile([128, COLS], bf, name=f"os{m}") for m in range(2)]
            whi = [[-1] * NBANK for _ in range(2)]
            for ki, k in enumerate(kact):
                last = ki == len(kact) - 1
                lo, hi, ob = LOc[k], HIc[k], offs[k]
                for m in range(2):
                    lhsT = ft_sb[:KDIM, k * C + m * 128: k * C + (m + 1) * 128]
                    for bk in range(lo // BANK, (hi - 1) // BANK + 1):
                        s = max(lo, bk * BANK)
                        e = min(hi, (bk + 1) * BANK)
                        is_last = k == last_k[bk]
                        # per-element has_written semantics: first write with
                        # start=True resets, later writes (start=False) add;
                        # stop is sim-only bookkeeping
                        if whi[m][bk] < 0:
                            pieces = [(s, e, True)]
                        elif SIM_SAFE:
                            pieces = []
                            if s < whi[m][bk]:
                                pieces.append((s, min(e, whi[m][bk]), False))
                            if e > whi[m][bk]:
                                pieces.append((max(s, whi[m][bk]), e, False))
                        else:
                            pieces = [(s, e, False)]
                        for pi, (ps, pe, st) in enumerate(pieces):
                            nc.tensor.matmul(
                                ptiles[m][bk][:, ps - bk * BANK: pe - bk * BANK],
                                lhsT=lhsT,
                                rhs=b_sb[:KDIM, ob + ps - lo: ob + pe - lo],
                                start=st,
                                stop=is_last and pi == len(pieces) - 1,
                            )
                        whi[m][bk] = max(whi[m][bk], e)
                # eager per-bank evacuation (m=0 via vector+sync, m=1 via
                # scalar): each PSUM tile is read exactly once, right after
                # its last accumulating chunk — no read ever blocks a later
                # matmul, and output DMA overlaps the remaining compute.
                for m in range(2):
                    for bk in range(NBANK):
                        if last_k.get(bk) != k:
                            continue
                        b0, b1 = bk * BANK, min((bk + 1) * BANK, COLS)
                        dst = out_sb[m][:, b0:b1]
                        if m == 0:
                            nc.vector.tensor_copy(dst, ptiles[m][bk][:, :b1 - b0])
                        else:
                            nc.scalar.copy(dst, ptiles[m][bk][:, :b1 - b0])
                        eng = nc.sync if m == 0 else nc.scalar
                        eng.dma_start(out_ap[m * 128:(m + 1) * 128, b0:b1], dst)

    nc.compile()
    return nc


def kernel(features, rois):
    global LAST_RESULTS
    from concourse import bass_utils

    features = np.asarray(features, dtype=np.float32)
    rois = np.asarray(rois, dtype=np.float32)
    hp = _host_prep(features, rois)

    shape_key = (hp["NB"], hp["COLS"], hp["NBANK"],
                 tuple(hp["LOc"]), tuple(hp["HIc"]),
                 tuple(bool(a) for a in hp["active"]),
                 tuple(int(o) for o in hp["offs"]))
    nc = _kernel_cache.get(shape_key)
    if nc is None:
        nc = _build_bass(shape_key)
        _kernel_cache[shape_key] = nc

    in_maps = [{"ft": np.ascontiguousarray(hp["FT"][c]),
                "bb": np.ascontiguousarray(hp["B"][c])}
               for c in range(NCORES)]
    res = bass_utils.run_bass_kernel_spmd(nc, in_maps, core_ids=list(range(NCORES)))
    LAST_RESULTS = res

    # unpack: out_core[c_chan, col(slot,q)] -> final[r, c_chan, p, q]
    final = np.zeros((hp["R"], C, POOLED, POOLED), dtype=np.float32)
    for c in range(NCORES):
        out = np.asarray(res.results[c]["out"], dtype=np.float32)  # [C, COLS]
        wins = hp["core_wins"][c]
        if not wins:
            continue
        sl = hp["slot"][c]
        rgs = np.array([w[2] for w in wins])
        ps = np.array([w[3] for w in wins])
        cols = out.reshape(C, -1, POOLED)[:, sl, :]    # [C, nwin, 7]
        final[rgs, :, ps, :] = cols.transpose(1, 0, 2)
    return final
